# revision 46
# baseline (speedup 1.0000x reference)
"""Trainium2 Bass kernel for nn_ByteToLatentAttention.

Sharding: 8 cores = 2 (batch) x 4 (head-groups of 4 heads).  Each core
computes a partial output  attn_part @ wout_rows + merged_raw_rows @ wbyp_rows
for its batch; the host sums the 4 partials per batch (/128 scale) and adds
wout_b.  No device collectives needed.

Precision: the bypass path (raw x @ wbyp), which dominates the output
magnitude, runs in fp32 (float32r matmuls).  The RMS-sum, K/Q/V projections
and the attn-out projection run in fp8e4m3 DoubleRow matmuls (2 k-tiles per
pass -> half the PE time); weights are pre-scaled x32 on the host so fp8
stays in its normal range, and the scale is divided back out via the
activation `scale` operand / host assembly (powers of two, exact).  The
attention core (scores, exp, attn@V) runs in bf16 with fp32 PSUM; a fraction
of the softmax exp tiles run on the Vector engine with a Schraudolph-style
int16/bf16 bit trick (attention contributes ~2% of output magnitude).

Self-contained: hardcodes all shapes; uses only numpy + concourse.
"""

from contextlib import ExitStack

import numpy as np

import concourse.bass as bass
import concourse.tile as tile
from concourse import bacc
from concourse import mybir
from concourse.bass_utils import run_bass_kernel_spmd

# ---- problem constants ----
B, S, D = 2, 4096, 512
BPL, H, DQK = 4, 16, 64
DLAT = 1024
LQ = S // BPL  # 1024
EPS = 1.1920929e-07
ROPE_BASE = 10000.0
NCORES = 8
NH = (H // 4) * DQK  # 256 features per core (4 heads)
P = 128

F32 = mybir.dt.float32
BF16 = mybir.dt.bfloat16
F8 = mybir.dt.float8e4
I16 = mybir.dt.int16
MM_F32 = mybir.dt.float32r  # full-rate PE path for 4-byte data

AF = mybir.ActivationFunctionType
ALU = mybir.AluOpType
AX = mybir.AxisListType
DR = mybir.MatmulPerfMode.DoubleRow

# fp8 weight pre-scale (host multiplies weights by WSCALE; device divides it
# back out via activation scale / host assembly).  Power of two => exact.
WSCALE = 32.0
W_INV = 1.0 / WSCALE
VEVAC = 0.125          # Vn = pv * 0.125 -> V x4 total
OUT_SCALE = 128.0      # out_partial = 128 * true partial (4 * 32)

# Schraudolph exp-via-int16-bits constants (round-to-nearest convert)
SCH_A = (1 << 7) / np.log(2.0)
SCH_B = 127.0 * (1 << 7) - 0.043677448 * (1 << 7)


def _kernel_body(ctx: ExitStack, tc, io):
    nc = tc.nc

    const = ctx.enter_context(tc.tile_pool(name="const", bufs=1))
    work = ctx.enter_context(tc.tile_pool(name="work", bufs=2))
    evp = ctx.enter_context(tc.tile_pool(name="evp", bufs=4))
    stage = ctx.enter_context(tc.tile_pool(name="stage", bufs=1))
    psP = ctx.enter_context(tc.tile_pool(name="psP", bufs=2, space="PSUM"))
    psS = ctx.enter_context(tc.tile_pool(name="psS", bufs=2, space="PSUM"))
    psAcc = ctx.enter_context(tc.tile_pool(name="psAcc", bufs=1, space="PSUM"))
    psDen = ctx.enter_context(tc.tile_pool(name="psDen", bufs=1, space="PSUM"))

    # ------- DMAs (HWDGE is FIFO per engine: issue in first-use order) -------
    ones128 = const.tile([P, P], BF16)
    nc.vector.memset(ones128, 1.0)
    ones64 = const.tile([P, 64], BF16)
    nc.vector.memset(ones64, 1.0)
    ones2f8 = const.tile([P, 2, P], F8)
    nc.vector.memset(ones2f8, 1.0)
    eps_sb = const.tile([P, 1], F32)
    nc.vector.memset(eps_sb, EPS)
    rot_sb = const.tile([P, P], BF16)
    nc.sync.dma_start(out=rot_sb, in_=io["rotm"])
    bq_sb = const.tile([P, 2], F32)
    nc.sync.dma_start(out=bq_sb, in_=io["bq"])
    bk_sb = const.tile([P, 2], F32)
    nc.sync.dma_start(out=bk_sb, in_=io["bk"])
    bv_sb = const.tile([P, 2], F32)
    nc.sync.dma_start(out=bv_sb, in_=io["bv"])

    xT = stage.tile([P, 4, S], BF16, tag="A")  # [d_p, dc, s] host-transposed

    def xchunk_dma(c8):
        # x_b is chunk-major [8, P, 4, 512] so each chunk is one contiguous
        # 512KB DRAM read
        ssl = slice(c8 * 512, (c8 + 1) * 512)
        nc.sync.dma_start(out=xT[:, :, ssl], in_=io["x_b"][c8])

    xchunk_dma(0)
    xchunk_dma(1)
    wk_sb = const.tile([P, 4, NH], F8)
    nc.sync.dma_start(out=wk_sb, in_=io["wk"])
    cs_k = const.tile([P, 2, S], BF16)
    nc.sync.dma_start(out=cs_k[:, 0, :], in_=io["cosk"])
    nc.sync.dma_start(out=cs_k[:, 1, :], in_=io["sink"])
    cosk_sb = cs_k[:, 0, :]
    sink_sb = cs_k[:, 1, :]
    wv_sb = const.tile([P, 4, NH], F8)
    nc.sync.dma_start(out=wv_sb, in_=io["wv"])
    xchunk_dma(2)
    xchunk_dma(3)
    wq_sb = const.tile([P, 16, NH], F8)
    nc.sync.dma_start(out=wq_sb, in_=io["wq"])
    cs_q = const.tile([P, 2, LQ], BF16)
    nc.sync.dma_start(out=cs_q[:, 0, :], in_=io["cosq"])
    nc.sync.dma_start(out=cs_q[:, 1, :], in_=io["sinq"])
    cosq_sb = cs_q[:, 0, :]
    sinq_sb = cs_q[:, 1, :]
    for c8 in range(4, 8):
        xchunk_dma(c8)
    wo_sb = const.tile([P, 2, DLAT], F8)
    nc.sync.dma_start(out=wo_sb, in_=io["wo"])
    wb_sb = const.tile([P, 4, DLAT], MM_F32)
    nc.sync.dma_start(out=wb_sb, in_=io["wb"])
    # bypT reuses xT's staging space (tag "A"): its DMA starts only after the
    # last rms chunk has consumed xT, well before out-projection needs it.
    bypT = stage.tile([P, 4, LQ], MM_F32, tag="A")
    nc.sync.dma_start(out=bypT, in_=io["x_byp"])

    # persistent big tensors
    normXT = const.tile([P, 4, S], F8)  # [d_p, dc, s] normalized x^T (fp8)
    QTr = const.tile([P, 2, LQ], BF16)  # roped Q^T (pair hp -> heads 2hp,2hp+1)
    KTr = const.tile([P, 2, S], BF16)  # roped K^T
    Vn = const.tile([P, 32, NH], BF16)  # [s_p, sc, n]  V natural (x4 scale)
    acT = const.tile([P, 2, LQ], F8)  # attn output^T (+v bias), x4 scale

    # ---------- PE warm-up (HAM un-throttle, runs during x DMA) ----------
    # enough back-to-back matmuls to hold the PE clock ramped until the
    # first x chunk lands (~5us after the warm-up starts)
    warm_ps = psS.tile([P, 1024], F32, tag="sc")
    for wi in range(48):
        nc.tensor.matmul(
            warm_ps[:, 0:128], lhsT=ones128, rhs=ones128, start=True, stop=True
        )

    # ---------------- phase functions ----------------
    def rms_chunk(c8):
        ssl = slice(c8 * 512, (c8 + 1) * 512)
        sq = work.tile([P, 4, 512], F8, tag="sq")
        for dc in range(4):
            if dc < 2:
                nc.scalar.square(out=sq[:, dc, :], in_=xT[:, dc, ssl])
            else:
                nc.vector.tensor_mul(
                    out=sq[:, dc, :], in0=xT[:, dc, ssl], in1=xT[:, dc, ssl]
                )
        pss = psP.tile([P, 512], F32, tag="mm")
        for m in range(2):
            nc.tensor.matmul(
                pss, lhsT=ones2f8, rhs=sq[:, 2 * m : 2 * m + 2, :],
                start=(m == 0), stop=(m == 1), perf_mode=DR,
            )
        rmsb = work.tile([P, 512], F32, tag="rmsb")
        nc.scalar.activation(
            out=rmsb, in_=pss, func=AF.Sqrt, bias=eps_sb, scale=1.0 / D
        )
        rinvf = work.tile([P, 512], F32, tag="rinvf")
        nc.vector.reciprocal_approx_fast(out=rinvf, in_=rmsb)
        rinvb = work.tile([P, 512], BF16, tag="rinvb")
        nc.vector.tensor_copy(out=rinvb, in_=rinvf)
        for dc in range(4):
            nc.vector.tensor_mul(
                out=normXT[:, dc, ssl], in0=xT[:, dc, ssl], in1=rinvb
            )

    def k_chunk(sf):
        # both DoubleRow projections back-to-back (one PE mode run), then the
        # rot matmuls, then the rope elementwise tail per half.
        ssl = slice(sf * 512, (sf + 1) * 512)
        ck = cosk_sb[:, ssl]
        sk = sink_sb[:, ssl]
        pks, kbs = [], []
        for mk in range(2):
            pk = psP.tile([P, 512], F32, tag="mm")
            for m in range(2):
                nc.tensor.matmul(
                    pk,
                    lhsT=wk_sb[:, 2 * m : 2 * m + 2, mk * P : (mk + 1) * P],
                    rhs=normXT[:, 2 * m : 2 * m + 2, ssl],
                    start=(m == 0),
                    stop=(m == 1),
                    perf_mode=DR,
                )
            pks.append(pk)
        for mk in range(2):
            kb = work.tile([P, 512], BF16, tag="qb", name=f"kb{sf}{mk}")
            nc.scalar.activation(
                out=kb, in_=pks[mk], func=AF.Identity, bias=bk_sb[:, mk : mk + 1],
                scale=W_INV,
            )
            kbs.append(kb)
        for mk in range(2):
            pr = psP.tile([P, 512], F32, tag="mm")
            nc.tensor.matmul(pr, lhsT=rot_sb, rhs=kbs[mk], start=True, stop=True)
            t1 = work.tile([P, 512], BF16, tag="t1")
            nc.vector.tensor_mul(out=t1, in0=kbs[mk], in1=ck)
            t2 = work.tile([P, 512], BF16, tag="t2")
            nc.vector.tensor_mul(out=t2, in0=pr, in1=sk)
            nc.vector.tensor_add(out=KTr[:, mk, ssl], in0=t1, in1=t2)

    def q_chunk(qf):
        qsl = slice(qf * 512, (qf + 1) * 512)
        nrs = normXT.rearrange("p c (m s) -> p c s m", s=4)
        pqs, qbs = [], []
        for mq in range(2):
            pq = psP.tile([P, 512], F32, tag="mm")
            for sub in range(4):
                for m in range(2):
                    kc = sub * 4 + 2 * m
                    nc.tensor.matmul(
                        pq,
                        lhsT=wq_sb[:, kc : kc + 2, mq * P : (mq + 1) * P],
                        rhs=nrs[:, 2 * m : 2 * m + 2, sub, qsl],
                        start=(sub == 0 and m == 0),
                        stop=(sub == 3 and m == 1),
                        perf_mode=DR,
                    )
            pqs.append(pq)
        for mq in range(2):
            qb = work.tile([P, 512], BF16, tag="qb", name=f"qb{qf}{mq}")
            nc.scalar.activation(
                out=qb, in_=pqs[mq], func=AF.Identity, bias=bq_sb[:, mq : mq + 1],
                scale=W_INV,
            )
            qbs.append(qb)
        for mq in range(2):
            pr = psP.tile([P, 512], F32, tag="mm")
            nc.tensor.matmul(pr, lhsT=rot_sb, rhs=qbs[mq], start=True, stop=True)
            t1 = work.tile([P, 512], BF16, tag="t1")
            nc.vector.tensor_mul(out=t1, in0=qbs[mq], in1=cosq_sb[:, qsl])
            t2 = work.tile([P, 512], BF16, tag="t2")
            nc.vector.tensor_mul(out=t2, in0=pr, in1=sinq_sb[:, qsl])
            nc.vector.tensor_add(out=QTr[:, mq, qsl], in0=t1, in1=t2)

    def v_chunk(sc):
        pv = psP.tile([P, NH], F32, tag="mm")
        for m in range(2):
            nc.tensor.matmul(
                pv,
                lhsT=normXT[:, 2 * m : 2 * m + 2, sc * P : (sc + 1) * P],
                rhs=wv_sb[:, 2 * m : 2 * m + 2, :],
                start=(m == 0),
                stop=(m == 1),
                perf_mode=DR,
            )
        nc.vector.tensor_scalar_mul(out=Vn[:, sc, :], in0=pv, scalar1=VEVAC)

    # attention state per (qc, hp): psum accumulators + esum pairing + exp tile
    att_state = {}

    def _exp_half(dst, src, on_dve):
        if on_dve:
            # Schraudolph fake-exp on the Vector engine (int16 bits -> bf16)
            nc.vector.tensor_scalar(
                out=dst.bitcast(I16), in0=src,
                scalar1=float(SCH_A * 0.125), scalar2=float(SCH_B),
                op0=ALU.mult, op1=ALU.add,
            )
        else:
            nc.scalar.activation(out=dst, in_=src, func=AF.Exp, scale=0.125)

    def attn_scores(qc, hp, sc):
        """Scores pair + whole-tile exp for iteration (qc, hp, sc).
        The exp runs on ONE engine per iteration (ACT mostly, DVE every
        4th): a single readiness edge keeps the downstream PE pairs
        adjacent, while consecutive iterations' exps overlap across the
        two engines."""
        qsl = slice(qc * 512, (qc + 1) * 512)
        ksl = slice(sc * P, (sc + 1) * P)
        psab = psS.tile([P, 1024], F32, tag="sc")
        nc.tensor.matmul(
            psab[:, 0:512], lhsT=KTr[0:64, hp, ksl], rhs=QTr[0:64, hp, qsl],
            start=True, stop=True, skip_group_check=True,
        )
        nc.tensor.matmul(
            psab[:, 512:1024], lhsT=KTr[64:128, hp, ksl], rhs=QTr[64:128, hp, qsl],
            start=True, stop=True, skip_group_check=True,
        )
        eab = evp.tile([P, 1024], BF16, tag="ea")
        _exp_half(eab, psab, sc % 4 == 3)
        att_state[(qc, hp, sc)] = eab

    def attn_av(qc, hp, sc):
        """attn@V for iteration (qc, hp, sc); emitted one iteration after
        its scores so the exp has a full iteration of slack before the
        in-order PE queue reaches these matmuls.  Odd sc also computes the
        esum (pair-sum of exp tiles) consumed by the delayed pden."""
        eab = att_state.pop((qc, hp, sc))
        ea = eab[:, 0:512]
        eb = eab[:, 512:1024]
        if sc == 0:
            att_state[(qc, hp, "pac")] = psAcc.tile([P, 512], F32, tag="pac",
                                                     name=f"pac{qc}{hp}")
            att_state[(qc, hp, "pden")] = psDen.tile([P, 512], F32, tag="pden",
                                                     name=f"pden{qc}{hp}")
        pac = att_state[(qc, hp, "pac")]
        st, sp = (sc == 0), (sc == S // P - 1)
        cA = slice((2 * hp) * 64, (2 * hp) * 64 + 64)
        cB = slice((2 * hp + 1) * 64, (2 * hp + 1) * 64 + 64)
        nc.tensor.matmul(
            pac[0:64, :], lhsT=Vn[:, sc, cA], rhs=ea,
            start=st, stop=sp, tile_position=(0, 0), skip_group_check=True,
        )
        nc.tensor.matmul(
            pac[64:128, :], lhsT=Vn[:, sc, cB], rhs=eb,
            start=st, stop=sp, tile_position=(0, 64), skip_group_check=True,
        )
        if sc % 2 == 0:
            att_state[(qc, hp, "eprev")] = eab
        else:
            eprev = att_state.pop((qc, hp, "eprev"))
            esum = evp.tile([P, 1024], BF16, tag="esum", name=f"es{qc}{hp}{sc}")
            nc.vector.tensor_add(out=esum, in0=eprev, in1=eab)
            att_state[(qc, hp, "esum", sc)] = esum

    def attn_pden(qc, hp, j):
        """Denominator matmuls for esum(j); emitted two iterations after the
        esum so even a GpSimd esum cannot stall the in-order PE queue."""
        esum = att_state.pop((qc, hp, "esum", j))
        pden = att_state[(qc, hp, "pden")]
        dst, dsp = (j == 1), (j == S // P - 1)
        nc.tensor.matmul(
            pden[0:64, :], lhsT=ones64, rhs=esum[:, 0:512],
            start=dst, stop=dsp, tile_position=(0, 0), skip_group_check=True,
        )
        nc.tensor.matmul(
            pden[64:128, :], lhsT=ones64, rhs=esum[:, 512:1024],
            start=dst, stop=dsp, tile_position=(0, 64), skip_group_check=True,
        )

    def attn_close(qc, hp):
        qsl = slice(qc * 512, (qc + 1) * 512)
        pac = att_state.pop((qc, hp, "pac"))
        pden = att_state.pop((qc, hp, "pden"))
        bc = work.tile([P, 512], F32, tag="bc")
        nc.vector.reciprocal_approx_fast(out=bc, in_=pden)
        tn = work.tile([P, 512], F32, tag="tn")
        nc.vector.tensor_mul(out=tn, in0=pac, in1=bc)
        nc.vector.tensor_scalar_add(
            out=acT[:, hp, qsl], in0=tn, scalar1=bv_sb[:, hp : hp + 1]
        )

    def outproj_q8(q8):
        # both DoubleRow attn-out matmuls first (one PE mode run), then the
        # fp32r bypass accumulations.
        qsl8 = slice(q8 * P, (q8 + 1) * P)
        osb = work.tile([P, DLAT], F32, tag="osb")
        pos = []
        for oc in range(2):
            osl = slice(oc * 512, (oc + 1) * 512)
            po = psP.tile([P, 512], F32, tag="mm")
            nc.tensor.matmul(
                po, lhsT=acT[:, :, qsl8], rhs=wo_sb[:, :, osl],
                start=True, stop=False, perf_mode=DR,
            )
            pos.append(po)
        for oc in range(2):
            osl = slice(oc * 512, (oc + 1) * 512)
            for dc in range(4):
                nc.tensor.matmul(
                    pos[oc],
                    lhsT=bypT[:, dc, qsl8],
                    rhs=wb_sb[:, dc, osl],
                    start=False,
                    stop=(dc == 3),
                )
            nc.vector.tensor_copy(out=osb[:, osl], in_=pos[oc])
            nc.sync.dma_start(out=io["out_partial"][qsl8, osl], in_=osb[:, osl])

    # ---------------- program order ----------------
    # Software-pipelined attention: scores(k+1) is emitted BEFORE the
    # exp-consuming body(k), so the PE streams the next scores pair while
    # the Scalar/Vector engine computes exp(k).  K/V/Q production and
    # the out-projection are hooked into the pipeline to fill PE gaps.
    iters = [(0, 0, sc) for sc in range(32)] + \
            [(0, 1, sc) for sc in range(32)] + \
            [(1, 0, sc) for sc in range(32)] + \
            [(1, 1, sc) for sc in range(32)]

    # hooks keyed by iteration index: emitted just before scores(k)
    pre_hooks = {}
    for sf in range(2, 8):
        hk = [("k", sf)]
        for sc in range(4 * sf, 4 * sf + 4):
            hk.append(("v", sc))
        pre_hooks[4 * sf] = hk
    pre_hooks[40] = [("q", 1)]
    # out-projection: qc0 chains during stream (1,0); qc1 chains at the end
    post_hooks = {66: [("o", 0)], 70: [("o", 1)], 74: [("o", 2)], 78: [("o", 3)]}

    def run_hook(h):
        kind, arg = h
        if kind == "rms":
            rms_chunk(arg)
        elif kind == "k":
            k_chunk(arg)
        elif kind == "v":
            v_chunk(arg)
        elif kind == "q":
            q_chunk(arg)
        elif kind == "o":
            outproj_q8(arg)

    # prologue: enough production for stream (0,0) to start; all rms chunks
    # run here so the ACT engine never reloads its exp table mid-stream.
    rms_chunk(0)
    rms_chunk(1)
    k_chunk(0)
    for sc in range(4):
        v_chunk(sc)
    rms_chunk(2)
    rms_chunk(3)
    k_chunk(1)
    for sc in range(4, 8):
        v_chunk(sc)
    q_chunk(0)
    rms_chunk(4)
    rms_chunk(5)
    rms_chunk(6)
    rms_chunk(7)

    # main loop (program order; the Tile scheduler overlaps by readiness).
    # Emission schedule per stream iteration sc: scores(sc) | AV(sc-1) |
    # pden(sc-3) — producers (exp on ACT/DVE, esum on DVE/GpSimd) get 1-2
    # iterations of slack before the in-order PE queue reaches the
    # consuming matmuls.
    def stream_epilogue(qc, hp):
        attn_av(qc, hp, 31)
        attn_pden(qc, hp, 27)
        attn_pden(qc, hp, 29)
        attn_pden(qc, hp, 31)
        attn_close(qc, hp)

    for k in range(len(iters)):
        qc, hp, sc = iters[k]
        for h in pre_hooks.get(k, ()):
            run_hook(h)
        attn_scores(qc, hp, sc)
        if sc == 0 and k > 0:
            stream_epilogue(iters[k - 1][0], iters[k - 1][1])
        if sc >= 1:
            attn_av(qc, hp, sc - 1)
        if sc >= 6 and sc % 2 == 0:
            attn_pden(qc, hp, sc - 5)
        for h in post_hooks.get(k, ()):
            run_hook(h)
    stream_epilogue(1, 1)
    for q8 in range(4, 8):
        outproj_q8(q8)


def build_program():
    nc = bacc.Bacc("TRN2", target_bir_lowering=False, debug=False)
    io = {}

    def inp(name, shape, dtype=F32):
        io[name] = nc.dram_tensor(name, list(shape), dtype, kind="ExternalInput").ap()

    inp("x_b", [8, P, 4, 512], BF16)
    inp("x_byp", [P, 4, LQ], MM_F32)
    inp("wq", [P, 16, NH], F8)
    inp("wk", [P, 4, NH], F8)
    inp("wv", [P, 4, NH], F8)
    inp("bq", [P, 2])
    inp("bk", [P, 2])
    inp("bv", [P, 2])
    inp("wo", [P, 2, DLAT], F8)
    inp("wb", [P, 4, DLAT], MM_F32)
    inp("cosq", [P, LQ], BF16)
    inp("sinq", [P, LQ], BF16)
    inp("cosk", [P, S], BF16)
    inp("sink", [P, S], BF16)
    inp("rotm", [P, P], BF16)
    io["out_partial"] = nc.dram_tensor(
        "out_partial", [LQ, DLAT], F32, kind="ExternalOutput"
    ).ap()

    with tile.TileContext(nc) as tc:
        with ExitStack() as ctx:
            _kernel_body(ctx, tc, io)
    nc.compile()
    return nc


def _chunked_rows(w, dtype):
    """[C*128, N] -> [128, C, N] (partition-major chunks for direct DMA)."""
    c = w.shape[0] // P
    return np.ascontiguousarray(w.reshape(c, P, -1).transpose(1, 0, 2).astype(dtype))


def _rope_tables(pos):
    half = DQK // 2
    invfreq = ROPE_BASE ** (-np.arange(half, dtype=np.float64) / half)
    ang = pos[:, None].astype(np.float64) * invfreq[None, :]
    cos = np.cos(ang)
    sin = np.sin(ang)
    cos64 = np.concatenate([cos, cos], axis=1).T  # [64, L]
    sin64 = np.concatenate([-sin, sin], axis=1).T
    cosT = np.concatenate([cos64, cos64], axis=0)
    sinT = np.concatenate([sin64, sin64], axis=0)
    return cosT, sinT


def _tf32(a):
    u = np.ascontiguousarray(np.asarray(a, dtype=np.float32)).view(np.uint32)
    lsb = (u >> np.uint32(13)) & np.uint32(1)
    u = (u + np.uint32(0x0FFF) + lsb) & np.uint32(0xFFFFE000)
    return u.view(np.float32)


def _bf16(a):
    import ml_dtypes

    return np.ascontiguousarray(np.asarray(a).astype(ml_dtypes.bfloat16))


def _f8(a):
    import ml_dtypes

    clipped = np.clip(np.asarray(a, dtype=np.float32), -240.0, 240.0)
    return np.ascontiguousarray(clipped.astype(ml_dtypes.float8_e4m3fn))


def make_in_map(core, inputs):
    b, hg = core // 4, core % 4
    x = np.asarray(inputs["x"], dtype=np.float32)
    nw = np.asarray(inputs["norm_w"], dtype=np.float32)
    wq_w = np.asarray(inputs["wq_w"], dtype=np.float32)
    wq_b = np.asarray(inputs["wq_b"], dtype=np.float32)
    wkv_w = np.asarray(inputs["wkv_w"], dtype=np.float32)
    wkv_b = np.asarray(inputs["wkv_b"], dtype=np.float32)
    wout_w = np.asarray(inputs["wout_w"], dtype=np.float32)
    wbyp_w = np.asarray(inputs["wbyp_w"], dtype=np.float32)

    nsl = slice(hg * NH, (hg + 1) * NH)
    vsl = slice(H * DQK + hg * NH, H * DQK + (hg + 1) * NH)
    wq_c = wq_w * np.tile(nw, BPL)[:, None]
    wkv_c = wkv_w * nw[:, None]

    cosq, sinq = _rope_tables(np.arange(LQ) * float(BPL))
    cosk, sink = _rope_tables(np.arange(S).astype(np.float64))

    rotm = np.zeros((P, P), dtype=np.float32)
    for m in range(P):
        blk, d = (m // 64) * 64, m % 64
        rotm[blk + (d + 32) % 64, m] = 1.0

    import ml_dtypes

    F8NP = ml_dtypes.float8_e4m3fn

    return {
        "x_b": _bf16(
            x[b].T.reshape(4, P, 8, 512).transpose(2, 1, 0, 3)
        ),
        "x_byp": _tf32(
            np.ascontiguousarray(x[b, hg::BPL, :].T.reshape(4, P, LQ).transpose(1, 0, 2))
        ),
        "wq": _f8(_chunked_rows(wq_c[:, nsl] * WSCALE, np.float32)),
        "wk": _f8(_chunked_rows(wkv_c[:, nsl] * WSCALE, np.float32)),
        "wv": _f8(_chunked_rows(wkv_c[:, vsl] * WSCALE, np.float32)),
        "bq": np.ascontiguousarray(wq_b[nsl].reshape(2, P).T),
        "bk": np.ascontiguousarray(wkv_b[nsl].reshape(2, P).T),
        # bias folded into acT at x4 scale (V path carries x4)
        "bv": np.ascontiguousarray(wkv_b[vsl].reshape(2, P).T) * 4.0,
        "wo": _f8(_chunked_rows(wout_w[nsl, :] * WSCALE, np.float32)),
        "wb": _tf32(_chunked_rows(wbyp_w[hg * D : (hg + 1) * D, :] * OUT_SCALE,
                                  np.float32)),
        "cosq": _bf16(cosq),
        "sinq": _bf16(sinq),
        "cosk": _bf16(cosk),
        "sink": _bf16(sink),
        "rotm": _bf16(rotm),
    }


_nc_cache = None


def _get_program():
    global _nc_cache
    if _nc_cache is None:
        _nc_cache = build_program()
    return _nc_cache


def run_device(inputs, trace=False):
    nc = _get_program()
    in_maps = [make_in_map(c, inputs) for c in range(NCORES)]
    res = run_bass_kernel_spmd(nc, in_maps, core_ids=list(range(NCORES)), trace=trace)
    return res


def assemble(parts, inputs):
    wout_b = np.asarray(inputs["wout_b"], dtype=np.float32)
    out = np.zeros((B, LQ, DLAT), dtype=np.float64)
    for c in range(NCORES):
        out[c // 4] += np.asarray(parts[c], dtype=np.float64)
    out *= 1.0 / OUT_SCALE
    out += wout_b[None, None, :].astype(np.float64)
    return out.astype(np.float32)


def kernel(**inputs):
    res = run_device(inputs)
    parts = [r["out_partial"] for r in res.results]
    return assemble(parts, inputs)
